# revision 6
# baseline (speedup 1.0000x reference)
"""Trainium2 Bass kernel for nn_NodeAttention (hypergraph message passing).

Math (reference):
    w      = sigmoid(x @ attn_w.T + attn_b)[:, 0]          # per-edge weight (M == N)
    e_feat = Binv * segsum_by_edge(x[node_idx]) @ lin_w.T  # node -> hyperedge
    D      = segsum_by_node(w[edge_idx])
    out    = Dinv * segsum_by_node(e_feat[edge_idx]) + bias

Distribution: 8 cores; core c owns edge rows [c*6250, (c+1)*6250) for the
node->edge phase and node rows of the same range for the edge->node phase.
Each phase is a row gather (SWDGE dma_gather from a replicated DRAM table)
followed by a one-hot-matmul segment sum over windows of 128 destination
segments. lin_w is applied once per 128-row window after aggregation (matmul
commutes with the segment sum); w is carried as column 128 of the intermediate
table so D falls out of the phase-B segment sum for free.

Both gather tables are bf16 (tolerance 2e-2 >> bf16 rounding):
  phase A gathers from xg = bf16(x), rows of 128 elems = 256B
  phase B gathers from ea [50000, 256] bf16 = [e_feat | w | pad], 512B rows

SWDGE descriptor generation (~7.5ns per static index on 2 Q7 cores) is the
bottleneck and its cost is per STATIC index (runtime -1 trimming only skips
descriptor emission, not index ingestion), so the tile counts are baked
per-window at compile time: window w uses max-over-cores ceil(count/128)
tiles for each index half (the program is SPMD so the shape must agree
across cores). Padding lanes use index 0 (a valid row) and are zeroed by the
dest one-hot.

Host-side work is limited to index preprocessing (partition by destination,
sort, pad) plus hyperedge degree counts — all derived from hyperedge_index
only — and the x -> bf16 input conversion. All x-dependent math runs on
device.
"""

import os
import sys
from contextlib import ExitStack

import numpy as np
import ml_dtypes

for _p in (
    "/root/.axon_site",
    "/root/.axon_site/_ro/trn_rl_repo",
    "/root/.axon_site/_ro/pypackages",
):
    if os.path.isdir(_p) and _p not in sys.path:
        sys.path.append(_p)

import concourse.bass as bass
import concourse.mybir as mybir
import concourse.tile as tile
from concourse import bacc
from concourse.bass_utils import run_bass_kernel_spmd
from concourse.masks import make_identity

P = 128
N_NODES = 50000
N_EDGES = 50000
C = 128          # feature channels
CT = 256         # bf16 intermediate row: [e_feat(128) | w(1) | pad(127)], 512B
HALF = 32768     # int16 index split point
NCORES = 8
SLAB = N_NODES // NCORES          # 6250 rows owned per core
WPC = (SLAB + P - 1) // P         # 49 windows of 128 destinations per core

F32 = mybir.dt.float32
BF16 = mybir.dt.bfloat16
I16 = mybir.dt.int16
NP_BF16 = ml_dtypes.bfloat16

# Set by test harness to capture NTFF profiles / exec times.
TRACE = False
LAST_EXEC_NS = {}

_PROGRAMS = {}

# One dma_gather call must write <= 4096 bytes per dst partition and <= 1024
# indices (HW packet limit, verified: 8 tiles x 512B rows passes, 8 x 768B
# and 12 x 512B abort).
MAX_GATHER_PART_BYTES = 4096
MAX_GATHER_IDXS = 1024


# ----------------------------------------------------------------------------
# Host-side index preprocessing
# ----------------------------------------------------------------------------

def _plan_phase(dst_ids, src_ids):
    """Group entries by (destination core, 128-dest window, src half); pad
    each group to the per-window max-over-cores tile count.

    Returns (tl, th, img_lo, img_hi, dst):
      tl, th: per-window tile counts (tuples of length WPC)
      img_lo: [NCORES, P, sum(tl) * 8] int16 dma_gather index image, packed
              by window
      img_hi: [NCORES, P, sum(th) * 8] int16 (indices rebased by -HALF)
      dst:    [NCORES, P, sum(tl + th)] fp32 dest-rel-to-window, pad -1,
              packed by window with lo tiles then hi tiles
    """
    dst_ids = np.asarray(dst_ids, np.int64)
    src_ids = np.asarray(src_ids, np.int64)
    core = dst_ids // SLAB
    local = dst_ids - core * SLAB
    w = local // P
    rel = (local - w * P).astype(np.float32)
    hi = (src_ids >= HALF).astype(np.int64)
    key = (core * WPC + w) * 2 + hi
    order = np.argsort(key, kind="stable")
    k = key[order]
    s = src_ids[order]
    r = rel[order]
    n_grp = NCORES * WPC * 2
    counts = np.bincount(k, minlength=n_grp).reshape(NCORES, WPC, 2)
    tiles = np.ceil(counts / P).astype(np.int64).max(axis=0)  # [WPC, 2]
    # >= 1 tile per window so the segment-sum matmul always initializes PSUM
    # (an all-(-1) dst column contributes zero)
    tiles[:, 0] = np.maximum(tiles[:, 0], 1)
    tl = tuple(int(v) for v in tiles[:, 0])
    th = tuple(int(v) for v in tiles[:, 1])

    starts = np.cumsum(counts.reshape(-1)) - counts.reshape(-1)
    rank = np.arange(k.shape[0], dtype=np.int64) - starts[k]
    half_flag = k % 2
    gw = k // 2
    cc = gw // WPC
    ww = gw - cc * WPC
    t_local = rank // P
    lane = rank - t_local * P

    # packed window offsets
    off_lo = np.concatenate([[0], np.cumsum(tiles[:, 0])])   # [WPC+1]
    off_hi = np.concatenate([[0], np.cumsum(tiles[:, 1])])
    t_tot_w = tiles.sum(axis=1)
    off_dst = np.concatenate([[0], np.cumsum(t_tot_w)])      # [WPC+1]

    # dst one-hot source: tile position within the packed dst image
    dst_tile = np.where(
        half_flag == 0,
        off_dst[ww] + t_local,
        off_dst[ww] + tiles[ww, 0] + t_local,
    )
    dst_img = np.full((NCORES, P, int(t_tot_w.sum())), -1.0, np.float32)
    dst_img[cc, lane, dst_tile] = r

    def build_img(sel, off_half, total_tiles, base):
        if total_tiles == 0:
            return np.zeros((NCORES, P, 0), np.int16)
        seq = np.zeros((NCORES, 16, total_tiles * 8), np.int16)
        # index position within window image: rank; global tile = off + t_local
        gtile = off_half[ww[sel]] + t_local[sel]
        pos = gtile * P + lane[sel]
        seq[cc[sel], pos % 16, pos // 16] = (s[sel] - base).astype(np.int16)
        return np.ascontiguousarray(np.tile(seq, (1, 8, 1)))

    img_lo = build_img(half_flag == 0, off_lo, int(tiles[:, 0].sum()), 0)
    img_hi = build_img(half_flag == 1, off_hi, int(tiles[:, 1].sum()), HALF)
    return tl, th, img_lo, img_hi, dst_img


def _to_bf16(a):
    return np.ascontiguousarray(np.asarray(a, np.float32).astype(NP_BF16))


# ----------------------------------------------------------------------------
# Bass programs
# ----------------------------------------------------------------------------

def _new_nc():
    return bacc.Bacc(
        "TRN2",
        target_bir_lowering=False,
        debug=False,
        enable_asserts=False,
        num_devices=NCORES,
    )


def _gather_window(nc, g, tables, imgs, offs, w, tl, th, ce, esz):
    """Issue the lo/hi dma_gathers for window w into tile g
    [P, (tl[w]+th[w])*ce], chunked per the per-call limits.
    offs = (off_lo, off_hi) packed tile offsets per window."""
    maxt = min(MAX_GATHER_PART_BYTES // (ce * esz), MAX_GATHER_IDXS // P)
    off = 0
    for half in (0, 1):
        t_half = (tl if half == 0 else th)[w]
        base_t = offs[half][w]
        img = imgs[half]
        tab = tables[half]
        t0 = 0
        while t0 < t_half:
            tn = min(maxt, t_half - t0)
            ni = tn * P
            nc.gpsimd.dma_gather(
                g[:, (off + t0) * ce : (off + t0 + tn) * ce].rearrange(
                    "p (t c) -> p t c", c=ce
                ),
                tab,
                img[:, (base_t + t0) * 8 : (base_t + t0 + tn) * 8],
                ni,
                ni,
                ce,
            )
            t0 += tn
        off += t_half


def _offsets(tl, th):
    off_lo, off_hi, off_dst = [], [], []
    a = b = d = 0
    for w in range(WPC):
        off_lo.append(a)
        off_hi.append(b)
        off_dst.append(d)
        a += tl[w]
        b += th[w]
        d += tl[w] + th[w]
    return off_lo, off_hi, off_dst, a, b, d


def _phase_a_program(tl, th):
    """Node->edge aggregation, producing the per-core slab of the
    intermediate table ea[slab, CT] = bf16[Binv * segsum(x rows) @ lin_w.T | w]."""
    off_lo, off_hi, off_dst, n_lo, n_hi, n_tot = _offsets(tl, th)
    tmax = max(tl[w] + th[w] for w in range(WPC))
    nc = _new_nc()
    xg = nc.dram_tensor("xg", [N_NODES, C], BF16, kind="ExternalInput").ap()
    xslab = nc.dram_tensor("xslab", [WPC * P, C], F32, kind="ExternalInput").ap()
    ilo = nc.dram_tensor("ilo", [P, n_lo * 8], I16, kind="ExternalInput").ap()
    ihi = nc.dram_tensor("ihi", [P, n_hi * 8], I16, kind="ExternalInput").ap()
    dst = nc.dram_tensor("dst", [P, n_tot], F32, kind="ExternalInput").ap()
    binv = nc.dram_tensor("binv", [P, WPC], F32, kind="ExternalInput").ap()
    wt = nc.dram_tensor("wt", [C, C], F32, kind="ExternalInput").ap()
    arep = nc.dram_tensor("arep", [P, C], F32, kind="ExternalInput").ap()
    bcol = nc.dram_tensor("bcol", [P, 1], F32, kind="ExternalInput").ap()
    eslab = nc.dram_tensor("eslab", [SLAB, CT], BF16, kind="ExternalOutput").ap()

    with tile.TileContext(nc) as tc:
        with ExitStack() as ctx:
            const = ctx.enter_context(tc.tile_pool(name="const", bufs=1))
            gpool = ctx.enter_context(tc.tile_pool(name="gather", bufs=3))
            spool = ctx.enter_context(tc.tile_pool(name="onehot", bufs=6))
            wpool = ctx.enter_context(tc.tile_pool(name="work", bufs=3))
            opool = ctx.enter_context(tc.tile_pool(name="out", bufs=3))
            pseg = ctx.enter_context(tc.tile_pool(name="pseg", bufs=2, space="PSUM"))
            ptr = ctx.enter_context(tc.tile_pool(name="ptr", bufs=2, space="PSUM"))
            pout = ctx.enter_context(tc.tile_pool(name="pout", bufs=2, space="PSUM"))

            ident = const.tile([P, P], F32)
            make_identity(nc, ident[:])
            iota_i = const.tile([P, P], mybir.dt.int32)
            nc.gpsimd.iota(iota_i[:], pattern=[[1, P]], base=0, channel_multiplier=0)
            iota_f = const.tile([P, P], F32)
            nc.vector.tensor_copy(iota_f[:], iota_i[:])

            wt_sb = const.tile([C, C], F32)
            nc.sync.dma_start(out=wt_sb[:], in_=wt[:])
            a_sb = const.tile([P, C], F32)
            nc.sync.dma_start(out=a_sb[:], in_=arep[:])
            b_sb = const.tile([P, 1], F32)
            nc.sync.dma_start(out=b_sb[:], in_=bcol[:])
            ilo_sb = const.tile([P, n_lo * 8], I16)
            nc.sync.dma_start(out=ilo_sb[:], in_=ilo[:])
            ihi_sb = const.tile([P, n_hi * 8], I16)
            nc.sync.dma_start(out=ihi_sb[:], in_=ihi[:])
            dst_sb = const.tile([P, n_tot], F32)
            nc.sync.dma_start(out=dst_sb[:], in_=dst[:])
            binv_sb = const.tile([P, WPC], F32)
            nc.sync.dma_start(out=binv_sb[:], in_=binv[:])

            # slab rows of x, window-major: xsl[p, w*C + c] = xslab[w*128 + p, c]
            xsl = const.tile([P, WPC * C], F32)
            nc.sync.dma_start(
                out=xsl[:].rearrange("p (w c) -> p w c", c=C),
                in_=xslab.rearrange("(w p) c -> p w c", p=P),
            )

            # attention scores for the slab: w = sigmoid(x . a + b), one col/window
            wraw = const.tile([P, WPC], F32)
            for w in range(WPC):
                prod = wpool.tile([P, C], F32, tag="prod")
                nc.vector.tensor_tensor(
                    prod[:], xsl[:, w * C : (w + 1) * C], a_sb[:],
                    op=mybir.AluOpType.mult,
                )
                nc.vector.tensor_reduce(
                    wraw[:, w : w + 1], prod[:],
                    axis=mybir.AxisListType.X, op=mybir.AluOpType.add,
                )
            wall = const.tile([P, WPC], F32)
            nc.scalar.activation(
                wall[:], wraw[:], mybir.ActivationFunctionType.Sigmoid,
                bias=b_sb[:, 0:1], scale=1.0,
            )

            for w in range(WPC):
                rows = min(P, SLAB - w * P)
                t_tot = tl[w] + th[w]
                g = gpool.tile([P, tmax * C], BF16, tag="g")
                _gather_window(
                    nc, g, (xg[:HALF, :], xg[HALF:, :]), (ilo_sb, ihi_sb),
                    (off_lo, off_hi), w, tl, th, C, 2,
                )
                ps = pseg.tile([P, C], F32)
                for t in range(t_tot):
                    col = off_dst[w] + t
                    s_t = spool.tile([P, P], BF16, tag="S")
                    nc.vector.tensor_tensor(
                        s_t[:],
                        dst_sb[:, col : col + 1].to_broadcast([P, P]),
                        iota_f[:],
                        op=mybir.AluOpType.is_equal,
                    )
                    nc.tensor.matmul(
                        out=ps[:], lhsT=s_t[:], rhs=g[:, t * C : (t + 1) * C],
                        start=(t == 0), stop=(t == t_tot - 1),
                    )
                # scale rows by Binv while draining PSUM
                epre = wpool.tile([P, C], F32, tag="epre")
                nc.scalar.activation(
                    epre[:], ps[:], mybir.ActivationFunctionType.Copy,
                    scale=binv_sb[:, w : w + 1],
                )
                pst = ptr.tile([P, P], F32)
                nc.tensor.transpose(pst[:], epre[:], ident[:])
                epret = wpool.tile([P, P], F32, tag="epret")
                nc.scalar.copy(epret[:], pst[:])
                pso = pout.tile([P, C], F32)
                nc.tensor.matmul(
                    out=pso[:], lhsT=epret[:], rhs=wt_sb[:], start=True, stop=True
                )
                ot = opool.tile([P, CT], BF16, tag="ot")
                nc.scalar.copy(ot[:, 0:C], pso[:])
                nc.vector.tensor_copy(ot[:, C : C + 1], wall[:, w : w + 1])
                nc.sync.dma_start(
                    out=eslab[w * P : w * P + rows, :], in_=ot[:rows, :]
                )
    nc.compile()
    return nc


def _phase_b_program(tl, th):
    """Edge->node aggregation over the full intermediate table, producing the
    per-core output slab out[slab, C] = Dinv * segsum(ea rows)[:, :C] + bias."""
    off_lo, off_hi, off_dst, n_lo, n_hi, n_tot = _offsets(tl, th)
    tmax = max(tl[w] + th[w] for w in range(WPC))
    nc = _new_nc()
    ea = nc.dram_tensor("ea", [N_EDGES, CT], BF16, kind="ExternalInput").ap()
    ilo = nc.dram_tensor("ilo", [P, n_lo * 8], I16, kind="ExternalInput").ap()
    ihi = nc.dram_tensor("ihi", [P, n_hi * 8], I16, kind="ExternalInput").ap()
    dst = nc.dram_tensor("dst", [P, n_tot], F32, kind="ExternalInput").ap()
    biasr = nc.dram_tensor("biasr", [P, C], F32, kind="ExternalInput").ap()
    outslab = nc.dram_tensor("outslab", [SLAB, C], F32, kind="ExternalOutput").ap()

    with tile.TileContext(nc) as tc:
        with ExitStack() as ctx:
            const = ctx.enter_context(tc.tile_pool(name="const", bufs=1))
            gpool = ctx.enter_context(tc.tile_pool(name="gather", bufs=3))
            spool = ctx.enter_context(tc.tile_pool(name="onehot", bufs=6))
            wpool = ctx.enter_context(tc.tile_pool(name="work", bufs=3))
            opool = ctx.enter_context(tc.tile_pool(name="out", bufs=3))
            pseg = ctx.enter_context(tc.tile_pool(name="pseg", bufs=2, space="PSUM"))

            iota_i = const.tile([P, P], mybir.dt.int32)
            nc.gpsimd.iota(iota_i[:], pattern=[[1, P]], base=0, channel_multiplier=0)
            iota_f = const.tile([P, P], F32)
            nc.vector.tensor_copy(iota_f[:], iota_i[:])

            bias_sb = const.tile([P, C], F32)
            nc.sync.dma_start(out=bias_sb[:], in_=biasr[:])
            ilo_sb = const.tile([P, n_lo * 8], I16)
            nc.sync.dma_start(out=ilo_sb[:], in_=ilo[:])
            ihi_sb = const.tile([P, n_hi * 8], I16)
            nc.sync.dma_start(out=ihi_sb[:], in_=ihi[:])
            dst_sb = const.tile([P, n_tot], F32)
            nc.sync.dma_start(out=dst_sb[:], in_=dst[:])

            for w in range(WPC):
                rows = min(P, SLAB - w * P)
                t_tot = tl[w] + th[w]
                g = gpool.tile([P, tmax * CT], BF16, tag="g")
                _gather_window(
                    nc, g, (ea[:HALF, :], ea[HALF:, :]), (ilo_sb, ihi_sb),
                    (off_lo, off_hi), w, tl, th, CT, 2,
                )
                ps = pseg.tile([P, C + 4], F32)
                for t in range(t_tot):
                    col = off_dst[w] + t
                    s_t = spool.tile([P, P], BF16, tag="S")
                    nc.vector.tensor_tensor(
                        s_t[:],
                        dst_sb[:, col : col + 1].to_broadcast([P, P]),
                        iota_f[:],
                        op=mybir.AluOpType.is_equal,
                    )
                    nc.tensor.matmul(
                        out=ps[:, : C + 1],
                        lhsT=s_t[:],
                        rhs=g[:, t * CT : t * CT + C + 1],
                        start=(t == 0), stop=(t == t_tot - 1),
                    )
                # Dinv = 1 / max(D, tiny); zero-degree rows have zero sums so
                # huge * 0 = 0 matches the reference's where(D > 0, 1/D, 0).
                dmax = wpool.tile([P, 1], F32, tag="dmax")
                nc.vector.tensor_scalar_max(dmax[:], ps[:, C : C + 1], 1e-30)
                dinv = wpool.tile([P, 1], F32, tag="dinv")
                nc.vector.reciprocal(dinv[:], dmax[:])
                ot = opool.tile([P, C], F32, tag="ot")
                nc.scalar.activation(
                    ot[:], ps[:, 0:C], mybir.ActivationFunctionType.Copy,
                    scale=dinv[:, 0:1],
                )
                nc.vector.tensor_tensor(
                    ot[:], ot[:], bias_sb[:], op=mybir.AluOpType.add
                )
                nc.sync.dma_start(
                    out=outslab[w * P : w * P + rows, :], in_=ot[:rows, :]
                )
    nc.compile()
    return nc


def _program(phase, tl, th):
    key = (phase, tl, th)
    if key not in _PROGRAMS:
        _PROGRAMS[key] = (
            _phase_a_program(tl, th) if phase == "A" else _phase_b_program(tl, th)
        )
    return _PROGRAMS[key]


# ----------------------------------------------------------------------------
# Entry point
# ----------------------------------------------------------------------------

def _run(nc, in_maps, label):
    kwargs = {}
    if TRACE:
        kwargs = dict(trace=True, trace_cores=[0])
    res = run_bass_kernel_spmd(nc, in_maps, core_ids=list(range(NCORES)), **kwargs)
    if res.exec_time_ns is not None:
        LAST_EXEC_NS[label] = res.exec_time_ns
    return res.results


def kernel(x, hyperedge_index, attn_w, attn_b, lin_w, bias):
    x = np.ascontiguousarray(np.asarray(x, dtype=np.float32))
    he = np.asarray(hyperedge_index)
    node_idx = he[0].astype(np.int64)
    edge_idx = he[1].astype(np.int64)
    attn_w = np.asarray(attn_w, dtype=np.float32)
    attn_b = np.asarray(attn_b, dtype=np.float32)
    lin_w = np.asarray(lin_w, dtype=np.float32)
    bias = np.asarray(bias, dtype=np.float32)

    # --- host index preprocessing ------------------------------------------
    a_tl, a_th, a_img_lo, a_img_hi, a_dst = _plan_phase(edge_idx, node_idx)
    b_tl, b_th, b_img_lo, b_img_hi, b_dst = _plan_phase(node_idx, edge_idx)

    bdeg = np.bincount(edge_idx, minlength=N_EDGES).astype(np.float32)
    binv_full = np.where(bdeg > 0, 1.0 / np.maximum(bdeg, 1.0), 0.0).astype(
        np.float32
    )
    pad = WPC * P - SLAB
    binv_cores = np.pad(
        binv_full.reshape(NCORES, SLAB), ((0, 0), (0, pad))
    ).reshape(NCORES, WPC, P).transpose(0, 2, 1)  # [NCORES, P, WPC]
    binv_cores = np.ascontiguousarray(binv_cores)

    wt_host = np.ascontiguousarray(lin_w.T)  # [in_ch, out_ch]
    a_rep = np.ascontiguousarray(np.broadcast_to(attn_w.reshape(1, C), (P, C)))
    b_col = np.full((P, 1), float(attn_b.reshape(-1)[0]), np.float32)
    bias_rep = np.ascontiguousarray(np.broadcast_to(bias.reshape(1, C), (P, C)))

    xg = _to_bf16(x)

    xslab_pad = np.zeros((NCORES, WPC * P, C), np.float32)
    xslab_pad[:, :SLAB] = x.reshape(NCORES, SLAB, C)

    # --- phase A: node -> edge ---------------------------------------------
    nc_a = _program("A", a_tl, a_th)
    in_maps_a = [
        {
            "xg": xg,
            "xslab": xslab_pad[c],
            "ilo": a_img_lo[c],
            "ihi": a_img_hi[c],
            "dst": a_dst[c],
            "binv": binv_cores[c],
            "wt": wt_host,
            "arep": a_rep,
            "bcol": b_col,
        }
        for c in range(NCORES)
    ]
    res_a = _run(nc_a, in_maps_a, "A")
    ea = np.ascontiguousarray(
        np.concatenate([np.asarray(r["eslab"]) for r in res_a], axis=0)
    )  # [N_EDGES, CT] bf16

    # --- phase B: edge -> node ---------------------------------------------
    nc_b = _program("B", b_tl, b_th)
    in_maps_b = [
        {
            "ea": ea,
            "ilo": b_img_lo[c],
            "ihi": b_img_hi[c],
            "dst": b_dst[c],
            "biasr": bias_rep,
        }
        for c in range(NCORES)
    ]
    res_b = _run(nc_b, in_maps_b, "B")
    out = np.concatenate([np.asarray(r["outslab"]) for r in res_b], axis=0)
    return np.ascontiguousarray(out.astype(np.float32))
